# revision 2
# baseline (speedup 1.0000x reference)
"""Causal self-attention (b=4, t=2048, c=1024, 16 heads x 64) on 8 Trainium2
NeuronCores via Bass/Tile.

Sharding: core j -> batch j//2, head-group g=j%2 (8 heads each). Each core
computes qkv for its heads, causal attention, and a partial output projection
(its heads' rows of W_proj); the host sums the two partials per batch and adds
b_proj.

Device kernel design:
- All matmul operands are fp16 (same PE streaming rate as bf16, 8x finer
  mantissa); PSUM accumulation is fp32. Max rel err vs fp32 reference ~4e-4.
- x arrives pre-transposed (xT [c, t]); qT/kT are produced with head-dim on
  partitions (lhsT = W), v with time on partitions (lhsT = xT slices). W_v is
  augmented with a 65th zero column per head whose bias is 1.0, so the AV
  matmul's PSUM row 64 accumulates sum(exp) = the softmax denominator.
- Attention runs on S^T blocks [tk=128, tq=512]: S^T = kT_h.T @ qT_h (row
  tile_position packs even/odd heads into array halves), exp on ScalarE with
  the 1/sqrt(d) scale fused, tk-block-pair-wide (1024) exp ops, AV accumulates
  over tk blocks. Causality: blocks above the diagonal are skipped, diagonal
  blocks compute only the valid tq range plus one [128,128] triangular mask
  multiply on VectorE.
- Normalization: reciprocal of the denominator row, broadcast across 64
  partitions via a DRAM-bounce DMA (K=1 matmul broadcast on the last chunk's
  critical tail), in-place multiply.
- Scheduling: ScalarE exp (~175us total) is the pacer of the attention
  pipeline while the PE's independent work (qkv of future chunks, projection
  of finished chunks) is drained as single-matmul "filler" thunks between
  attention blocks. K/V (and Q) production for ALL chunks is queued up front
  so late, exp-heavy chunks overlap the early PE-heavy phase; per-chunk drain
  markers guarantee q(ch) is emitted before attention of chunk ch.
"""

import numpy as np

N_CORES = 8
B, T, C = 4, 2048, 1024
NH, HD = 16, 64
HPC = 8
HCOLS = HPC * HD  # 512
VAUG = HPC * (HD + 1)  # 520
TCH = 512
NCH = T // TCH
SCALE = 1.0 / np.sqrt(HD)

_CACHE = {}
LAST_RESULTS = None


def _build_nc():
    from concourse import bacc
    import concourse.mybir as mybir
    import concourse.tile as tile

    f32 = mybir.dt.float32
    bf16 = mybir.dt.float16  # fp16: same PE rate as bf16, 8x better mantissa
    fp16 = mybir.dt.float16
    Alu = mybir.AluOpType
    Act = mybir.ActivationFunctionType

    nc = bacc.Bacc("TRN2", target_bir_lowering=False, debug=False, num_devices=N_CORES)

    xT_d = nc.dram_tensor("xT", [C, T], bf16, kind="ExternalInput")
    Wq_d = nc.dram_tensor("Wq", [C, HCOLS], bf16, kind="ExternalInput")
    Wk_d = nc.dram_tensor("Wk", [C, HCOLS], bf16, kind="ExternalInput")
    Wv_d = nc.dram_tensor("Wv", [C, VAUG], bf16, kind="ExternalInput")
    Wp_d = nc.dram_tensor("Wp", [HCOLS, C], bf16, kind="ExternalInput")
    bq_d = nc.dram_tensor("bq", [HCOLS], f32, kind="ExternalInput")
    bk_d = nc.dram_tensor("bk", [HCOLS], f32, kind="ExternalInput")
    bv_d = nc.dram_tensor("bv", [1, VAUG], f32, kind="ExternalInput")
    tri_d = nc.dram_tensor("TRI", [128, 128], bf16, kind="ExternalInput")
    out_d = nc.dram_tensor("out", [T, C], f32, kind="ExternalOutput")

    KS = C // 128  # 8
    MS = HCOLS // 128  # 4
    TSUB = TCH // 128  # 4

    with tile.TileContext(nc) as tc:
        with (
            tc.tile_pool(name="persist", bufs=1) as persist,
            tc.tile_pool(name="stream", bufs=2) as stream,
            tc.tile_pool(name="es_pool", bufs=10) as esp,
            tc.tile_pool(name="small", bufs=5) as small,
            tc.tile_pool(name="pA", bufs=2, space="PSUM") as pA,
            tc.tile_pool(name="pS", bufs=2, space="PSUM") as pS,
            tc.tile_pool(name="pY", bufs=2, space="PSUM") as pY,
            tc.tile_pool(name="dram", bufs=8, space="DRAM") as dram,
        ):
            bqk = persist.tile([128, 2 * MS], f32, tag="bqk")
            bq = bqk[:, 0:MS]
            bk = bqk[:, MS:2 * MS]
            nc.sync.dma_start(bq, bq_d.ap().rearrange("(m p) -> p m", p=128))
            nc.sync.dma_start(bk, bk_d.ap().rearrange("(m p) -> p m", p=128))
            tri = persist.tile([128, 128], bf16, tag="tri")
            nc.sync.dma_start(tri[:], tri_d.ap())
            bv_row = persist.tile([1, VAUG], f32, tag="bv_row")
            nc.sync.dma_start(bv_row[:], bv_d.ap())
            ones1 = persist.tile([1, 128], f32, tag="ones1")
            nc.vector.memset(ones1[:], 1.0)
            ones1h = persist.tile([1, 64], fp16, tag="ones1h")
            with nc.allow_low_precision(reason="exact 1.0"):
                nc.vector.tensor_copy(ones1h[:], ones1[0:1, 0:64])

            Wk = persist.tile([128, KS, HCOLS], bf16, tag="Wk")
            Wv = persist.tile([128, KS, VAUG], bf16, tag="Wv")
            Wq = persist.tile([128, KS, HCOLS], bf16, tag="Wq")
            Wp = persist.tile([128, MS, C], bf16, tag="Wp")

            kT = persist.tile([128, MS, T], bf16, tag="kT")
            v = persist.tile([128, T // 128, VAUG], bf16, tag="v")

            bvb = persist.tile([128, VAUG], f32, tag="bvb")

            def make_xt(ch):
                xt = stream.tile([128, KS, TCH], bf16, tag="xt", bufs=4, name="xt")
                xsrc = (xT_d.ap()[:, ch * TCH:(ch + 1) * TCH]
                        .rearrange("(ko p) t -> p ko t", p=128))
                for kk in range(KS):
                    nc.sync.dma_start(xt[:, kk:kk + 1, :], xsrc[:, kk:kk + 1, :])
                return xt

            def group(thunks, scope, mm_fn, evac_fn, n_mm=KS):
                st = {}

                def mk(k):
                    def t():
                        with nc.named_scope(scope):
                            if k == 0:
                                st["ps"] = pA.tile([128, 512], f32, tag="pA",
                                                   name="psa")
                            mm_fn(st["ps"], k)
                            if k == n_mm - 1:
                                evac_fn(st["ps"])
                    return t
                thunks.extend(mk(k) for k in range(n_mm))

            def kv_thunks(ch, xt):
                """k and v production for chunk ch (reads xt)."""
                tsl = slice(ch * TCH, (ch + 1) * TCH)
                thunks = []
                sc = f"A{ch}"

                def k_mm(m):
                    def mm(ps, k):
                        nc.tensor.matmul(ps[:], Wk[:, k, m * 128:(m + 1) * 128],
                                         xt[:, k, :], start=(k == 0),
                                         stop=(k == KS - 1))

                    def ev(ps):
                        with nc.allow_low_precision(reason="bf16"):
                            nc.vector.tensor_scalar(
                                out=kT[:, m, tsl], in0=ps[:],
                                scalar1=bk[:, m:m + 1], scalar2=None, op0=Alu.add)
                    group(thunks, sc, mm, ev)

                def v_mm(ts):
                    tk_i = ch * TSUB + ts
                    xsl = xt[:, :, ts * 128:(ts + 1) * 128]

                    def mm(ps, k):
                        nc.tensor.matmul(ps[:], xsl[:, k, :], Wv[:, k, 0:512],
                                         start=(k == 0), stop=(k == KS - 1))

                    def ev(ps):
                        with nc.allow_low_precision(reason="bf16"):
                            nc.vector.tensor_tensor(
                                v[:, tk_i, 0:512], ps[:], bvb[:, 0:512], Alu.add)
                    group(thunks, sc, mm, ev)

                    def mm8(ps, k):
                        nc.tensor.matmul(ps[:, 0:8], xsl[:, k, :], Wv[:, k, 512:520],
                                         start=(k == 0), stop=(k == KS - 1))

                    def ev8(ps):
                        with nc.allow_low_precision(reason="bf16"):
                            nc.vector.tensor_tensor(
                                v[:, tk_i, 512:520], ps[:, 0:8], bvb[:, 512:520],
                                Alu.add)
                    group(thunks, sc, mm8, ev8)

                for m in range(MS):
                    k_mm(m)
                for ts in range(TSUB):
                    v_mm(ts)
                return thunks

            def q_thunks(ch, xt):
                qt = stream.tile([128, MS, TCH], bf16, tag="qt", bufs=4, name="qt")
                thunks = []
                sc = f"A{ch}"

                def q_mm(m):
                    def mm(ps, k):
                        nc.tensor.matmul(ps[:], Wq[:, k, m * 128:(m + 1) * 128],
                                         xt[:, k, :], start=(k == 0),
                                         stop=(k == KS - 1))

                    def ev(ps):
                        with nc.allow_low_precision(reason="bf16"):
                            nc.vector.tensor_scalar(
                                out=qt[:, m, :], in0=ps[:],
                                scalar1=bq[:, m:m + 1], scalar2=None, op0=Alu.add)
                    group(thunks, sc, mm, ev)

                for m in range(MS):
                    q_mm(m)
                return qt, thunks

            fill_q = []
            popped = [0]
            queued = [0]

            def drain(n):
                while n > 0 and fill_q:
                    fill_q.pop(0)()
                    popped[0] += 1
                    n -= 1

            def drain_until(mark):
                while popped[0] < mark and fill_q:
                    fill_q.pop(0)()
                    popped[0] += 1

            def queue(thunks):
                fill_q.extend(thunks)
                queued[0] += len(thunks)

            def emit_pair(ch, qt, yt, p, per_drain, mm_bcast=False):
                nblk = (ch + 1) * TSUB
                hA, hB = 2 * p, 2 * p + 1
                sc = f"attn{ch}"
                psy = {h: pY.tile([128, 512], f32, tag="pY", name=f"psy{h}")
                       for h in (hA, hB)}
                for g in range(nblk // 2):
                    i0, i1 = 2 * g, 2 * g + 1
                    pss = {}
                    # S matmuls: explicit row tile_position for pair concurrency
                    with nc.named_scope(sc):
                        for half, i in ((0, i0), (1, i1)):
                            dk = i - ch * TSUB
                            vs = 128 * dk if dk > 0 else 0
                            for h in (hA, hB):
                                pb = (h % 2) * 64
                                hm = h // 2
                                if half == 0:
                                    pss[h] = pS.tile([128, 1024], f32, tag="pS",
                                                     name=f"pss{h}")
                                nc.tensor.matmul(
                                    pss[h][:, half * TCH + vs: (half + 1) * TCH],
                                    kT[pb:pb + 64, hm, i * 128:(i + 1) * 128],
                                    qt[pb:pb + 64, hm, vs:TCH],
                                    start=True, stop=True,
                                    tile_position=(pb, 0))
                    drain(per_drain)
                    es = {}
                    with nc.named_scope(sc):
                        for h in (hA, hB):
                            es[h] = esp.tile([128, 2 * TCH], bf16, tag="es",
                                             name=f"es{h}")
                            dk1 = i1 - ch * TSUB
                            if dk1 <= 0:
                                with nc.allow_low_precision(reason="bf16"):
                                    nc.scalar.activation(es[h][:, :], pss[h][:, :],
                                                         Act.Exp, scale=float(SCALE))
                            else:
                                for half, i in ((0, i0), (1, i1)):
                                    dk = i - ch * TSUB
                                    vs = 128 * dk if dk > 0 else 0
                                    sl = slice(half * TCH + vs, (half + 1) * TCH)
                                    with nc.allow_low_precision(reason="bf16"):
                                        nc.scalar.activation(es[h][:, sl], pss[h][:, sl],
                                                             Act.Exp, scale=float(SCALE))
                            for half, i in ((0, i0), (1, i1)):
                                dk = i - ch * TSUB
                                if dk >= 0:
                                    vs = 128 * dk
                                    sl = slice(half * TCH + vs, half * TCH + vs + 128)
                                    with nc.allow_low_precision(reason="bf16"):
                                        nc.vector.tensor_tensor(
                                            es[h][:, sl], es[h][:, sl], tri[:], Alu.mult)
                        for half, i in ((0, i0), (1, i1)):
                            dk = i - ch * TSUB
                            vs = 128 * dk if dk > 0 else 0
                            for h in (hA, hB):
                                nc.tensor.matmul(
                                    psy[h][0:65, vs:TCH],
                                    v[:, i, h * 65:(h + 1) * 65],
                                    es[h][:, half * TCH + vs: (half + 1) * TCH],
                                    start=(i == 0), stop=(i == nblk - 1))
                    drain(per_drain)
                # normalization: reciprocal row -> DRAM bounce -> broadcast DMA
                with nc.named_scope(f"norm{ch}"):
                    for h in (hA, hB):
                        pb = (h % 2) * 64
                        hm = h // 2
                        lrow = small.tile([1, TCH], f32, tag="lrow", name=f"lrow{h}")
                        nc.vector.tensor_copy(lrow[0:1, :], psy[h][64:65, :])
                        rrow = small.tile([1, TCH], f32, tag="rrow")
                        nc.vector.reciprocal_approx_fast(rrow[0:1, :], lrow[0:1, :])
                        with nc.allow_low_precision(reason="bf16"):
                            # plain evac first so the psum bank frees fast
                            nc.vector.tensor_copy(yt[pb:pb + 64, hm, :], psy[h][0:64, :])
                        if mm_bcast:
                            # low-latency path for the final chunk: K=1 fp32 matmul
                            # broadcast (no DRAM roundtrip on the critical tail)
                            psb = pA.tile([128, 512], f32, tag="pA", name=f"psb{h}")
                            nc.tensor.matmul(psb[0:64, :], ones1[0:1, 0:64],
                                             rrow[0:1, :], start=True, stop=True)
                            with nc.allow_low_precision(reason="bf16"):
                                nc.vector.tensor_tensor(
                                    yt[pb:pb + 64, hm, :], yt[pb:pb + 64, hm, :],
                                    psb[0:64, :], Alu.mult)
                        else:
                            drow = dram.tile([1, TCH], f32, tag="drow", name=f"drow{h}")
                            nc.sync.dma_start(drow[:], rrow[:])
                            rbc = small.tile([128, TCH], f32, tag="rbc", name=f"rbc{h}")
                            nc.sync.dma_start(rbc[pb:pb + 64, :],
                                              drow[0:1, :].to_broadcast([64, TCH]))
                            with nc.allow_low_precision(reason="bf16"):
                                nc.vector.tensor_tensor(
                                    yt[pb:pb + 64, hm, :], yt[pb:pb + 64, hm, :],
                                    rbc[pb:pb + 64, :], Alu.mult)

            def proj_thunks(ch, yt):
                tq0 = ch * TCH
                thunks = []
                sc = f"proj{ch}"
                for mt in range(TSUB):
                    for n in range(C // 512):
                        st = {}

                        def mk(k, mt=mt, n=n, st=st):
                            def t():
                                with nc.named_scope(sc):
                                    if k == 3:
                                        st["ps"] = pA.tile([128, 512], f32, tag="pA",
                                                           name="psp")
                                    nc.tensor.matmul(
                                        st["ps"][:],
                                        yt[:, k, mt * 128:(mt + 1) * 128],
                                        Wp[:, k, n * 512:(n + 1) * 512],
                                        start=(k == 3), stop=(k == MS - 2))
                                    if k == MS - 2:
                                        ot = small.tile([128, 512], f32, tag="ot")
                                        nc.vector.tensor_copy(ot[:], st["ps"][:])
                                        nc.sync.dma_start(
                                            out_d.ap()[tq0 + mt * 128:
                                                       tq0 + (mt + 1) * 128,
                                                       n * 512:(n + 1) * 512],
                                            ot[:])
                            return t
                        thunks.extend(mk(k) for k in (3, 0, 1, 2))
                return thunks

            # ---- main schedule ----
            # chunk-0 inputs first so the PE can start ASAP
            xt0 = make_xt(0)
            wksrc = Wk_d.ap().rearrange("(ko p) m -> p ko m", p=128)
            for kk in range(KS):
                nc.sync.dma_start(Wk[:, kk:kk + 1, :], wksrc[:, kk:kk + 1, :])
            nc.sync.dma_start(Wv[:], Wv_d.ap().rearrange("(ko p) m -> p ko m", p=128))
            nc.sync.dma_start(Wq[:], Wq_d.ap().rearrange("(ko p) m -> p ko m", p=128))
            nc.sync.dma_start(Wp[:], Wp_d.ap().rearrange("(m p) e -> p m e", p=128))

            # bvb broadcast setup (PE warms up while the weight DMAs stream)
            ps = pA.tile([128, 512], f32, tag="pA")
            nc.tensor.matmul(ps[:, :], ones1[0:1, :], bv_row[0:1, 0:512],
                             start=True, stop=True)
            nc.scalar.copy(bvb[:, 0:512], ps[:, :])
            ps = pA.tile([128, 512], f32, tag="pA")
            nc.tensor.matmul(ps[:, 0:8], ones1[0:1, :], bv_row[0:1, 512:520],
                             start=True, stop=True)
            nc.scalar.copy(bvb[:, 512:520], ps[:, 0:8])

            # chunk 0 phase A emitted directly
            for t in kv_thunks(0, xt0):
                t()
            qt0, q0 = q_thunks(0, xt0)
            for t in q0:
                t()

            # queue phase A of all later chunks up front: kv then q per chunk
            qt_tab = {0: qt0}
            q_marker = {0: 0}
            for c in range(1, NCH):
                xtc = make_xt(c)
                queue(kv_thunks(c, xtc))
                qtc, qc = q_thunks(c, xtc)
                queue(qc)
                qt_tab[c] = qtc
                q_marker[c] = queued[0]

            def n_drains(c):
                return 2 * 4 * ((c + 1) * TSUB // 2)

            yt_tab = {}
            for ch in range(NCH):
                drain_until(q_marker[ch])
                yt = stream.tile([128, MS, TCH], bf16, tag="yt", bufs=4, name="yt")
                yt_tab[ch] = yt
                if ch < NCH - 1:
                    pts = sum(n_drains(c) for c in range(ch, NCH - 1))
                else:
                    pts = n_drains(ch)
                per_drain = max(1, -(-len(fill_q) // max(pts, 1)))
                for p in (3, 0, 1, 2):
                    emit_pair(ch, qt_tab[ch], yt, p, per_drain,
                              mm_bcast=(ch == NCH - 1))
                if ch < NCH - 1:
                    queue(proj_thunks(ch, yt))
            drain(len(fill_q))
            for t in proj_thunks(NCH - 1, yt_tab[NCH - 1]):
                t()

    nc.compile()
    return nc


def _get_nc():
    if "nc" not in _CACHE:
        _CACHE["nc"] = _build_nc()
    return _CACHE["nc"]


def kernel(x, W_qkv, b_qkv, W_proj, b_proj):
    global LAST_RESULTS
    from concourse.bass_utils import run_bass_kernel_spmd

    x = np.asarray(x, dtype=np.float32)
    W_qkv = np.asarray(W_qkv, dtype=np.float32)
    b_qkv = np.asarray(b_qkv, dtype=np.float32)
    W_proj = np.asarray(W_proj, dtype=np.float32)
    b_proj = np.asarray(b_proj, dtype=np.float32)

    nc = _get_nc()

    tri = np.tril(np.ones((128, 128), dtype=np.float32)).T.copy()  # tri[p,f]=1 iff p<=f

    in_maps = []
    for j in range(N_CORES):
        bi, g = j // 2, j % 2
        c0 = g * HCOLS
        Wv_h = W_qkv[:, 2 * C + c0:2 * C + c0 + HCOLS]
        bv_h = b_qkv[2 * C + c0:2 * C + c0 + HCOLS]
        Wv_aug = np.zeros((C, VAUG), dtype=np.float32)
        bv_aug = np.zeros((1, VAUG), dtype=np.float32)
        for h in range(HPC):
            Wv_aug[:, h * 65:h * 65 + 64] = Wv_h[:, h * 64:(h + 1) * 64]
            bv_aug[0, h * 65:h * 65 + 64] = bv_h[h * 64:(h + 1) * 64]
            bv_aug[0, h * 65 + 64] = 1.0
        bf16 = np.float16
        in_maps.append({
            "xT": x[bi].T.astype(bf16),
            "Wq": W_qkv[:, c0:c0 + HCOLS].astype(bf16),
            "Wk": W_qkv[:, C + c0:C + c0 + HCOLS].astype(bf16),
            "Wv": Wv_aug.astype(bf16),
            "Wp": W_proj[c0:c0 + HCOLS, :].astype(bf16),
            "bq": np.ascontiguousarray(b_qkv[c0:c0 + HCOLS]),
            "bk": np.ascontiguousarray(b_qkv[C + c0:C + c0 + HCOLS]),
            "bv": bv_aug,
            "TRI": tri.astype(bf16),
        })

    res = run_bass_kernel_spmd(nc, in_maps, list(range(N_CORES)))
    LAST_RESULTS = res

    out = np.empty((B, T, C), dtype=np.float32)
    for bi in range(B):
        out[bi] = res.results[2 * bi]["out"] + res.results[2 * bi + 1]["out"] + b_proj
    return out


# revision 7
# speedup vs baseline: 1.0209x; 1.0209x over previous
"""Causal self-attention (b=4, t=2048, c=1024, 16 heads x 64) on 8 Trainium2
NeuronCores via Bass/Tile.

Sharding: core j -> batch j//2, head-group g=j%2 (8 heads each). Each core
computes qkv for its heads, causal attention, and a partial output projection
(its heads' rows of W_proj); the host sums the two partials per batch and adds
b_proj.

Device kernel design:
- All matmul operands are fp16 (same PE streaming rate as bf16, 8x finer
  mantissa); PSUM accumulation is fp32. Max rel err vs fp32 reference ~4e-4.
- x arrives pre-transposed (xT [c, t]); qT/kT are produced with head-dim on
  partitions (lhsT = W), v with time on partitions (lhsT = xT slices). W_v is
  augmented with a 65th zero column per head whose bias is 1.0, so the AV
  matmul's PSUM row 64 accumulates sum(exp) = the softmax denominator.
- Attention runs on S^T blocks [tk=128, tq=512]: S^T = kT_h.T @ qT_h (row
  tile_position packs even/odd heads into array halves), exp on ScalarE with
  the 1/sqrt(d) scale fused, tk-block-pair-wide (1024) exp ops, AV accumulates
  over tk blocks. Causality: blocks above the diagonal are skipped, diagonal
  blocks compute only the valid tq range plus one [128,128] triangular mask
  multiply on VectorE.
- Normalization: reciprocal of the denominator row, broadcast across 64
  partitions via a DRAM-bounce DMA (K=1 matmul broadcast on the last chunk's
  critical tail), in-place multiply.
- Scheduling: ScalarE exp (~175us total) is the pacer of the attention
  pipeline while the PE's independent work (qkv of future chunks, projection
  of finished chunks) is drained as single-matmul "filler" thunks between
  attention blocks. K/V (and Q) production for ALL chunks is queued up front
  so late, exp-heavy chunks overlap the early PE-heavy phase; per-chunk drain
  markers guarantee q(ch) is emitted before attention of chunk ch.
"""

import numpy as np

N_CORES = 8
B, T, C = 4, 2048, 1024
NH, HD = 16, 64
HPC = 8
HCOLS = HPC * HD  # 512
VAUG = HPC * (HD + 1)  # 520
TCH = 512
NCH = T // TCH
SCALE = 1.0 / np.sqrt(HD)

_CACHE = {}
LAST_RESULTS = None


def _build_nc():
    from concourse import bacc
    import concourse.mybir as mybir
    import concourse.tile as tile

    f32 = mybir.dt.float32
    bf16 = mybir.dt.float16  # fp16: same PE rate as bf16, 8x better mantissa
    fp16 = mybir.dt.float16
    Alu = mybir.AluOpType
    Act = mybir.ActivationFunctionType

    nc = bacc.Bacc("TRN2", target_bir_lowering=False, debug=False, num_devices=N_CORES)

    xT_d = nc.dram_tensor("xT", [C, T], bf16, kind="ExternalInput")
    Wq_d = nc.dram_tensor("Wq", [C, HCOLS], bf16, kind="ExternalInput")
    Wk_d = nc.dram_tensor("Wk", [C, HCOLS], bf16, kind="ExternalInput")
    Wv_d = nc.dram_tensor("Wv", [C, VAUG], bf16, kind="ExternalInput")
    Wp_d = nc.dram_tensor("Wp", [HCOLS, C], bf16, kind="ExternalInput")
    bq_d = nc.dram_tensor("bq", [HCOLS], f32, kind="ExternalInput")
    bk_d = nc.dram_tensor("bk", [HCOLS], f32, kind="ExternalInput")
    bv_d = nc.dram_tensor("bv", [1, VAUG], f32, kind="ExternalInput")
    tri_d = nc.dram_tensor("TRI", [128, 128], bf16, kind="ExternalInput")
    out_d = nc.dram_tensor("out", [T, C], f32, kind="ExternalOutput")

    KS = C // 128  # 8
    MS = HCOLS // 128  # 4
    TSUB = TCH // 128  # 4

    with tile.TileContext(nc) as tc:
        with (
            tc.tile_pool(name="persist", bufs=1) as persist,
            tc.tile_pool(name="stream", bufs=2) as stream,
            tc.tile_pool(name="es_pool", bufs=10) as esp,
            tc.tile_pool(name="small", bufs=5) as small,
            tc.tile_pool(name="pA", bufs=2, space="PSUM") as pA,
            tc.tile_pool(name="pS", bufs=2, space="PSUM") as pS,
            tc.tile_pool(name="pY", bufs=2, space="PSUM") as pY,
            tc.tile_pool(name="dram", bufs=8, space="DRAM") as dram,
        ):
            bqk = persist.tile([128, 2 * MS], f32, tag="bqk")
            bq = bqk[:, 0:MS]
            bk = bqk[:, MS:2 * MS]
            nc.sync.dma_start(bq, bq_d.ap().rearrange("(m p) -> p m", p=128))
            nc.sync.dma_start(bk, bk_d.ap().rearrange("(m p) -> p m", p=128))
            tri = persist.tile([128, 128], bf16, tag="tri")
            nc.sync.dma_start(tri[:], tri_d.ap())
            bv_row = persist.tile([1, VAUG], f32, tag="bv_row")
            nc.sync.dma_start(bv_row[:], bv_d.ap())
            ones1 = persist.tile([1, 128], f32, tag="ones1")
            nc.vector.memset(ones1[:], 1.0)
            ones1h = persist.tile([1, 64], fp16, tag="ones1h")
            with nc.allow_low_precision(reason="exact 1.0"):
                nc.vector.tensor_copy(ones1h[:], ones1[0:1, 0:64])

            Wk = persist.tile([128, KS, HCOLS], bf16, tag="Wk")
            Wv = persist.tile([128, KS, VAUG], bf16, tag="Wv")
            Wq = persist.tile([128, KS, HCOLS], bf16, tag="Wq")
            Wp = persist.tile([128, MS, C], bf16, tag="Wp")

            kT = persist.tile([128, MS, T], bf16, tag="kT")
            v = persist.tile([128, T // 128, VAUG], bf16, tag="v")

            bvb = persist.tile([128, VAUG], f32, tag="bvb")

            def make_xt(ch):
                xt = stream.tile([128, KS, TCH], bf16, tag="xt", bufs=4, name="xt")
                xsrc = (xT_d.ap()[:, ch * TCH:(ch + 1) * TCH]
                        .rearrange("(ko p) t -> p ko t", p=128))
                for kk in range(KS):
                    nc.sync.dma_start(xt[:, kk:kk + 1, :], xsrc[:, kk:kk + 1, :])
                return xt

            def group(thunks, scope, mm_fn, evac_fn, n_mm=KS):
                st = {}

                def mk(k):
                    def t():
                        with nc.named_scope(scope):
                            if k == 0:
                                st["ps"] = pA.tile([128, 512], f32, tag="pA",
                                                   name="psa")
                            mm_fn(st["ps"], k)
                            if k == n_mm - 1:
                                evac_fn(st["ps"])
                    return t
                thunks.extend(mk(k) for k in range(n_mm))

            def kv_thunks(ch, xt):
                """k and v production for chunk ch (reads xt)."""
                tsl = slice(ch * TCH, (ch + 1) * TCH)
                thunks = []
                sc = f"A{ch}"

                def k_mm(m):
                    def mm(ps, k):
                        nc.tensor.matmul(ps[:], Wk[:, k, m * 128:(m + 1) * 128],
                                         xt[:, k, :], start=(k == 0),
                                         stop=(k == KS - 1))

                    def ev(ps):
                        with nc.allow_low_precision(reason="bf16"):
                            nc.vector.tensor_scalar(
                                out=kT[:, m, tsl], in0=ps[:],
                                scalar1=bk[:, m:m + 1], scalar2=None, op0=Alu.add)
                    group(thunks, sc, mm, ev)

                def v_mm(ts):
                    tk_i = ch * TSUB + ts
                    xsl = xt[:, :, ts * 128:(ts + 1) * 128]

                    def mm(ps, k):
                        nc.tensor.matmul(ps[:], xsl[:, k, :], Wv[:, k, 0:512],
                                         start=(k == 0), stop=(k == KS - 1))

                    def ev(ps):
                        with nc.allow_low_precision(reason="bf16"):
                            nc.vector.tensor_tensor(
                                v[:, tk_i, 0:512], ps[:], bvb[:, 0:512], Alu.add)
                    group(thunks, sc, mm, ev)

                    def mm8(ps, k):
                        nc.tensor.matmul(ps[:, 0:8], xsl[:, k, :], Wv[:, k, 512:520],
                                         start=(k == 0), stop=(k == KS - 1))

                    def ev8(ps):
                        with nc.allow_low_precision(reason="bf16"):
                            nc.vector.tensor_tensor(
                                v[:, tk_i, 512:520], ps[:, 0:8], bvb[:, 512:520],
                                Alu.add)
                    group(thunks, sc, mm8, ev8)

                for m in range(MS):
                    k_mm(m)
                for ts in range(TSUB):
                    v_mm(ts)
                return thunks

            def q_thunks(ch, xt):
                qt = stream.tile([128, MS, TCH], bf16, tag="qt", bufs=4, name="qt")
                thunks = []
                sc = f"A{ch}"

                def q_mm(m):
                    def mm(ps, k):
                        nc.tensor.matmul(ps[:], Wq[:, k, m * 128:(m + 1) * 128],
                                         xt[:, k, :], start=(k == 0),
                                         stop=(k == KS - 1))

                    def ev(ps):
                        with nc.allow_low_precision(reason="bf16"):
                            nc.vector.tensor_scalar(
                                out=qt[:, m, :], in0=ps[:],
                                scalar1=bq[:, m:m + 1], scalar2=None, op0=Alu.add)
                    group(thunks, sc, mm, ev)

                for m in range(MS):
                    q_mm(m)
                return qt, thunks

            fill_q = []
            proj_q = []
            allow_proj = [False]
            popped = [0]
            queued = [0]

            def drain(n):
                while n > 0 and fill_q:
                    fill_q.pop(0)()
                    popped[0] += 1
                    n -= 1
                while n > 0 and allow_proj[0] and proj_q:
                    proj_q.pop(0)()
                    n -= 1

            def drain_until(mark):
                while popped[0] < mark and fill_q:
                    fill_q.pop(0)()
                    popped[0] += 1

            def queue(thunks):
                fill_q.extend(thunks)
                queued[0] += len(thunks)

            def emit_pair(ch, qt, yt, p, per_drain):
                nblk = (ch + 1) * TSUB
                hA, hB = 2 * p, 2 * p + 1
                sc = f"attn{ch}"
                psy = {h: pY.tile([128, 512], f32, tag="pY", name=f"psy{h}")
                       for h in (hA, hB)}
                for g in range(nblk // 2):
                    i0, i1 = 2 * g, 2 * g + 1
                    pss = {}
                    # S matmuls: explicit row tile_position for pair concurrency
                    with nc.named_scope(sc):
                        for half, i in ((0, i0), (1, i1)):
                            dk = i - ch * TSUB
                            vs = 128 * dk if dk > 0 else 0
                            for h in (hA, hB):
                                pb = (h % 2) * 64
                                hm = h // 2
                                if half == 0:
                                    pss[h] = pS.tile([128, 1024], f32, tag="pS",
                                                     name=f"pss{h}")
                                nc.tensor.matmul(
                                    pss[h][:, half * TCH + vs: (half + 1) * TCH],
                                    kT[pb:pb + 64, hm, i * 128:(i + 1) * 128],
                                    qt[pb:pb + 64, hm, vs:TCH],
                                    start=True, stop=True,
                                    tile_position=(pb, 0))
                    drain(per_drain)
                    es = {}
                    with nc.named_scope(sc):
                        for h in (hA, hB):
                            es[h] = esp.tile([128, 2 * TCH], bf16, tag="es",
                                             name=f"es{h}")
                            dk1 = i1 - ch * TSUB
                            if dk1 <= 0:
                                with nc.allow_low_precision(reason="bf16"):
                                    nc.scalar.activation(es[h][:, :], pss[h][:, :],
                                                         Act.Exp, scale=float(SCALE))
                            else:
                                for half, i in ((0, i0), (1, i1)):
                                    dk = i - ch * TSUB
                                    vs = 128 * dk if dk > 0 else 0
                                    sl = slice(half * TCH + vs, (half + 1) * TCH)
                                    with nc.allow_low_precision(reason="bf16"):
                                        nc.scalar.activation(es[h][:, sl], pss[h][:, sl],
                                                             Act.Exp, scale=float(SCALE))
                            for half, i in ((0, i0), (1, i1)):
                                dk = i - ch * TSUB
                                if dk >= 0:
                                    vs = 128 * dk
                                    sl = slice(half * TCH + vs, half * TCH + vs + 128)
                                    with nc.allow_low_precision(reason="bf16"):
                                        nc.vector.tensor_tensor(
                                            es[h][:, sl], es[h][:, sl], tri[:], Alu.mult)
                        for half, i in ((0, i0), (1, i1)):
                            dk = i - ch * TSUB
                            vs = 128 * dk if dk > 0 else 0
                            for h in (hA, hB):
                                nc.tensor.matmul(
                                    psy[h][0:65, vs:TCH],
                                    v[:, i, h * 65:(h + 1) * 65],
                                    es[h][:, half * TCH + vs: (half + 1) * TCH],
                                    start=(i == 0), stop=(i == nblk - 1))
                    drain(per_drain)
                # normalization: reciprocal of the denominator row, broadcast
                # across 64 partitions via a K=1 fp32 matmul, in-place multiply
                with nc.named_scope(f"norm{ch}"):
                    for h in (hA, hB):
                        pb = (h % 2) * 64
                        hm = h // 2
                        lrow = small.tile([1, TCH], f32, tag="lrow", name=f"lrow{h}")
                        nc.vector.tensor_copy(lrow[0:1, :], psy[h][64:65, :])
                        rrow = small.tile([1, TCH], f32, tag="rrow")
                        nc.vector.reciprocal_approx_fast(rrow[0:1, :], lrow[0:1, :])
                        with nc.allow_low_precision(reason="bf16"):
                            # plain evac first so the psum bank frees fast
                            nc.vector.tensor_copy(yt[pb:pb + 64, hm, :], psy[h][0:64, :])
                        psb = pA.tile([128, 512], f32, tag="pA", name=f"psb{h}")
                        nc.tensor.matmul(psb[0:64, :], ones1[0:1, 0:64],
                                         rrow[0:1, :], start=True, stop=True)
                        with nc.allow_low_precision(reason="bf16"):
                            nc.vector.tensor_tensor(
                                yt[pb:pb + 64, hm, :], yt[pb:pb + 64, hm, :],
                                psb[0:64, :], Alu.mult)

            def proj_thunks(ch, yt):
                tq0 = ch * TCH
                thunks = []
                sc = f"proj{ch}"
                for mt in range(TSUB):
                    for n in range(C // 512):
                        st = {}

                        def mk(k, mt=mt, n=n, st=st):
                            def t():
                                with nc.named_scope(sc):
                                    if k == 3:
                                        st["ps"] = pA.tile([128, 512], f32, tag="pA",
                                                           name="psp")
                                    nc.tensor.matmul(
                                        st["ps"][:],
                                        yt[:, k, mt * 128:(mt + 1) * 128],
                                        Wp[:, k, n * 512:(n + 1) * 512],
                                        start=(k == 3), stop=(k == MS - 2))
                                    if k == MS - 2:
                                        ot = small.tile([128, 512], f32, tag="ot")
                                        nc.vector.tensor_copy(ot[:], st["ps"][:])
                                        nc.sync.dma_start(
                                            out_d.ap()[tq0 + mt * 128:
                                                       tq0 + (mt + 1) * 128,
                                                       n * 512:(n + 1) * 512],
                                            ot[:])
                            return t
                        thunks.extend(mk(k) for k in (3, 0, 1, 2))
                return thunks

            # ---- main schedule ----
            # chunk-0 inputs first so the PE can start ASAP: interleave the
            # xt(0) and Wk k-slices (the first k-matmul needs slice 0 of both)
            xt0 = stream.tile([128, KS, TCH], bf16, tag="xt", bufs=4, name="xt")
            x0src = xT_d.ap()[:, 0:TCH].rearrange("(ko p) t -> p ko t", p=128)
            wksrc = Wk_d.ap().rearrange("(ko p) m -> p ko m", p=128)
            for kk in range(KS):
                nc.sync.dma_start(xt0[:, kk:kk + 1, :], x0src[:, kk:kk + 1, :])
                nc.sync.dma_start(Wk[:, kk:kk + 1, :], wksrc[:, kk:kk + 1, :])
            nc.sync.dma_start(Wv[:], Wv_d.ap().rearrange("(ko p) m -> p ko m", p=128))
            nc.sync.dma_start(Wq[:], Wq_d.ap().rearrange("(ko p) m -> p ko m", p=128))
            nc.sync.dma_start(Wp[:], Wp_d.ap().rearrange("(m p) e -> p m e", p=128))

            # bvb broadcast setup (PE warms up while the weight DMAs stream)
            ps = pA.tile([128, 512], f32, tag="pA")
            nc.tensor.matmul(ps[:, :], ones1[0:1, :], bv_row[0:1, 0:512],
                             start=True, stop=True)
            nc.scalar.copy(bvb[:, 0:512], ps[:, :])
            ps = pA.tile([128, 512], f32, tag="pA")
            nc.tensor.matmul(ps[:, 0:8], ones1[0:1, :], bv_row[0:1, 512:520],
                             start=True, stop=True)
            nc.scalar.copy(bvb[:, 512:520], ps[:, 0:8])

            # chunk 0 phase A emitted directly
            for t in kv_thunks(0, xt0):
                t()
            qt0, q0 = q_thunks(0, xt0)
            for t in q0:
                t()

            # queue phase A of all later chunks up front: kv then q per chunk
            qt_tab = {0: qt0}
            q_marker = {0: 0}
            for c in range(1, NCH):
                xtc = make_xt(c)
                queue(kv_thunks(c, xtc))
                qtc, qc = q_thunks(c, xtc)
                queue(qc)
                qt_tab[c] = qtc
                q_marker[c] = queued[0]

            def n_drains(c):
                return 2 * 4 * ((c + 1) * TSUB // 2)

            # Pacing: land A(ch+1) ~60% into window ch (early chunks) so the
            # exp of later chunks can start sooner; spread A(3) over all of
            # window 2; drain the deferred proj fillers only during chunk 3,
            # where exp dominates and the PE would otherwise go sparse (HAM
            # re-throttles the PE clock on sparse activity).
            yt_tab = {}
            for ch in range(NCH):
                drain_until(q_marker[ch])
                yt = stream.tile([128, MS, TCH], bf16, tag="yt", bufs=4, name="yt")
                yt_tab[ch] = yt
                if ch == NCH - 1:
                    allow_proj[0] = True
                    todo = len(fill_q) + len(proj_q)
                    pts = n_drains(ch)
                else:
                    nxt = q_marker.get(ch + 1, queued[0])
                    todo = nxt - popped[0]
                    frac = 0.6 if ch < 2 else 1.0
                    pts = max(1, int(n_drains(ch) * frac))
                per_drain = max(1, -(-todo // max(pts, 1)))
                for p in (3, 0, 1, 2):
                    emit_pair(ch, qt_tab[ch], yt, p, per_drain)
                if ch < NCH - 1:
                    proj_q.extend(proj_thunks(ch, yt))
            drain(len(fill_q) + len(proj_q))
            for t in proj_thunks(NCH - 1, yt_tab[NCH - 1]):
                t()

    nc.compile()
    return nc


def _get_nc():
    if "nc" not in _CACHE:
        _CACHE["nc"] = _build_nc()
    return _CACHE["nc"]


def kernel(x, W_qkv, b_qkv, W_proj, b_proj):
    global LAST_RESULTS
    from concourse.bass_utils import run_bass_kernel_spmd

    x = np.asarray(x, dtype=np.float32)
    W_qkv = np.asarray(W_qkv, dtype=np.float32)
    b_qkv = np.asarray(b_qkv, dtype=np.float32)
    W_proj = np.asarray(W_proj, dtype=np.float32)
    b_proj = np.asarray(b_proj, dtype=np.float32)

    nc = _get_nc()

    tri = np.tril(np.ones((128, 128), dtype=np.float32)).T.copy()  # tri[p,f]=1 iff p<=f

    in_maps = []
    for j in range(N_CORES):
        bi, g = j // 2, j % 2
        c0 = g * HCOLS
        Wv_h = W_qkv[:, 2 * C + c0:2 * C + c0 + HCOLS]
        bv_h = b_qkv[2 * C + c0:2 * C + c0 + HCOLS]
        Wv_aug = np.zeros((C, VAUG), dtype=np.float32)
        bv_aug = np.zeros((1, VAUG), dtype=np.float32)
        for h in range(HPC):
            Wv_aug[:, h * 65:h * 65 + 64] = Wv_h[:, h * 64:(h + 1) * 64]
            bv_aug[0, h * 65:h * 65 + 64] = bv_h[h * 64:(h + 1) * 64]
            bv_aug[0, h * 65 + 64] = 1.0
        bf16 = np.float16
        in_maps.append({
            "xT": x[bi].T.astype(bf16),
            "Wq": W_qkv[:, c0:c0 + HCOLS].astype(bf16),
            "Wk": W_qkv[:, C + c0:C + c0 + HCOLS].astype(bf16),
            "Wv": Wv_aug.astype(bf16),
            "Wp": W_proj[c0:c0 + HCOLS, :].astype(bf16),
            "bq": np.ascontiguousarray(b_qkv[c0:c0 + HCOLS]),
            "bk": np.ascontiguousarray(b_qkv[C + c0:C + c0 + HCOLS]),
            "bv": bv_aug,
            "TRI": tri.astype(bf16),
        })

    res = run_bass_kernel_spmd(nc, in_maps, list(range(N_CORES)))
    LAST_RESULTS = res

    out = np.empty((B, T, C), dtype=np.float32)
    for bi in range(B):
        out[bi] = res.results[2 * bi]["out"] + res.results[2 * bi + 1]["out"] + b_proj
    return out


# revision 10
# speedup vs baseline: 1.3115x; 1.2846x over previous
"""Causal self-attention (b=4, t=2048, c=1024, 16 heads x 64) on 8 Trainium2
NeuronCores via Bass/Tile.

Sharding: core j -> batch j//2, head-group g=j%2 (8 heads each). Each core
computes qkv for its heads, causal attention, and a partial output projection
(its heads' rows of W_proj); the host sums the two partials per batch and adds
b_proj.

Device kernel design:
- All matmul operands are fp16 (same PE streaming rate as bf16, 8x finer
  mantissa); PSUM accumulation is fp32. Max rel err vs fp32 reference ~4e-4.
- x arrives pre-transposed (xT [c, t]); qT/kT are produced with head-dim on
  partitions (lhsT = W), v with time on partitions (lhsT = xT slices). W_v is
  augmented with a 65th zero column per head whose bias is 1.0, so the AV
  matmul's PSUM row 64 accumulates sum(exp) = the softmax denominator.
- Attention runs on S^T blocks [tk=128, tq=512]: S^T = kT_h.T @ qT_h (row
  tile_position packs even/odd heads into array halves), exp on ScalarE with
  the 1/sqrt(d) scale fused, tk-block-pair-wide (1024) exp ops, AV accumulates
  over tk blocks. Causality: blocks above the diagonal are skipped, diagonal
  blocks compute only the valid tq range plus one [128,128] triangular mask
  multiply on VectorE.
- Normalization: reciprocal of the denominator row, broadcast across 64
  partitions via a DRAM-bounce DMA (K=1 matmul broadcast on the last chunk's
  critical tail), in-place multiply.
- Scheduling: ScalarE exp (~175us total) is the pacer of the attention
  pipeline while the PE's independent work (qkv of future chunks, projection
  of finished chunks) is drained as single-matmul "filler" thunks between
  attention blocks. K/V (and Q) production for ALL chunks is queued up front
  so late, exp-heavy chunks overlap the early PE-heavy phase; per-chunk drain
  markers guarantee q(ch) is emitted before attention of chunk ch.
"""

import numpy as np

N_CORES = 8
B, T, C = 4, 2048, 1024
NH, HD = 16, 64
HPC = 8
HCOLS = HPC * HD  # 512
VAUG = HPC * (HD + 1)  # 520
TCH = 512
NCH = T // TCH
SCALE = 1.0 / np.sqrt(HD)

_CACHE = {}
LAST_RESULTS = None


def _build_nc():
    from concourse import bacc
    import concourse.mybir as mybir
    import concourse.tile as tile

    f32 = mybir.dt.float32
    bf16 = mybir.dt.float16  # fp16: same PE rate as bf16, 8x better mantissa
    fp16 = mybir.dt.float16
    Alu = mybir.AluOpType
    Act = mybir.ActivationFunctionType

    nc = bacc.Bacc("TRN2", target_bir_lowering=False, debug=False, num_devices=N_CORES)

    xT_d = nc.dram_tensor("xT", [C, T], bf16, kind="ExternalInput")
    Wq_d = nc.dram_tensor("Wq", [C, HCOLS], bf16, kind="ExternalInput")
    Wk_d = nc.dram_tensor("Wk", [C, HCOLS], bf16, kind="ExternalInput")
    Wv_d = nc.dram_tensor("Wv", [C, VAUG], bf16, kind="ExternalInput")
    Wp_d = nc.dram_tensor("Wp", [HCOLS, C], bf16, kind="ExternalInput")
    bq_d = nc.dram_tensor("bq", [HCOLS], f32, kind="ExternalInput")
    bk_d = nc.dram_tensor("bk", [HCOLS], f32, kind="ExternalInput")
    bv_d = nc.dram_tensor("bv", [1, VAUG], f32, kind="ExternalInput")
    tri_d = nc.dram_tensor("TRI", [128, 128], bf16, kind="ExternalInput")
    out_d = nc.dram_tensor("out", [T, C], f32, kind="ExternalOutput")

    KS = C // 128  # 8
    MS = HCOLS // 128  # 4
    TSUB = TCH // 128  # 4

    with tile.TileContext(nc) as tc:
        with (
            tc.tile_pool(name="persist", bufs=1) as persist,
            tc.tile_pool(name="stream", bufs=2) as stream,
            tc.tile_pool(name="es_pool", bufs=10) as esp,
            tc.tile_pool(name="small", bufs=5) as small,
            tc.tile_pool(name="pA", bufs=2, space="PSUM") as pA,
            tc.tile_pool(name="pS", bufs=2, space="PSUM") as pS,
            tc.tile_pool(name="pY", bufs=2, space="PSUM") as pY,
            tc.tile_pool(name="dram", bufs=8, space="DRAM") as dram,
        ):
            bqk = persist.tile([128, 2 * MS], f32, tag="bqk")
            bq = bqk[:, 0:MS]
            bk = bqk[:, MS:2 * MS]
            nc.sync.dma_start(bq, bq_d.ap().rearrange("(m p) -> p m", p=128))
            nc.sync.dma_start(bk, bk_d.ap().rearrange("(m p) -> p m", p=128))
            tri = persist.tile([128, 128], bf16, tag="tri")
            nc.sync.dma_start(tri[:], tri_d.ap())
            bv_row = persist.tile([1, VAUG], f32, tag="bv_row")
            nc.sync.dma_start(bv_row[:], bv_d.ap())
            ones1 = persist.tile([1, 128], f32, tag="ones1")
            nc.vector.memset(ones1[:], 1.0)
            ones1h = persist.tile([1, 64], fp16, tag="ones1h")
            with nc.allow_low_precision(reason="exact 1.0"):
                nc.vector.tensor_copy(ones1h[:], ones1[0:1, 0:64])

            Wk = persist.tile([128, KS, HCOLS], bf16, tag="Wk")
            Wv = persist.tile([128, KS, VAUG], bf16, tag="Wv")
            Wq = persist.tile([128, KS, HCOLS], bf16, tag="Wq")
            Wp = persist.tile([128, MS, C], bf16, tag="Wp")

            kT = persist.tile([128, MS, T], bf16, tag="kT")
            v = persist.tile([128, T // 128, VAUG], bf16, tag="v")

            bvb = persist.tile([128, VAUG], f32, tag="bvb")

            def make_xt(ch):
                xt = stream.tile([128, KS, TCH], bf16, tag="xt", bufs=4, name="xt")
                xsrc = (xT_d.ap()[:, ch * TCH:(ch + 1) * TCH]
                        .rearrange("(ko p) t -> p ko t", p=128))
                for kk in range(KS):
                    nc.sync.dma_start(xt[:, kk:kk + 1, :], xsrc[:, kk:kk + 1, :])
                return xt

            def group(thunks, scope, mm_fn, evac_fn, n_mm=KS):
                st = {}

                def mk(k):
                    def t():
                        with nc.named_scope(scope):
                            if k == 0:
                                st["ps"] = pA.tile([128, 512], f32, tag="pA",
                                                   name="psa")
                            mm_fn(st["ps"], k)
                            if k == n_mm - 1:
                                evac_fn(st["ps"])
                    return t
                thunks.extend(mk(k) for k in range(n_mm))

            def kv_thunks(ch, xt):
                """k and v production for chunk ch (reads xt)."""
                tsl = slice(ch * TCH, (ch + 1) * TCH)
                thunks = []
                sc = f"A{ch}"

                def k_mm(m):
                    def mm(ps, k):
                        nc.tensor.matmul(ps[:], Wk[:, k, m * 128:(m + 1) * 128],
                                         xt[:, k, :], start=(k == 0),
                                         stop=(k == KS - 1))

                    def ev(ps):
                        with nc.allow_low_precision(reason="bf16"):
                            nc.vector.tensor_scalar(
                                out=kT[:, m, tsl], in0=ps[:],
                                scalar1=bk[:, m:m + 1], scalar2=None, op0=Alu.add)
                    group(thunks, sc, mm, ev)

                def v_mm(ts):
                    tk_i = ch * TSUB + ts
                    xsl = xt[:, :, ts * 128:(ts + 1) * 128]

                    def mm(ps, k):
                        nc.tensor.matmul(ps[:], xsl[:, k, :], Wv[:, k, 0:512],
                                         start=(k == 0), stop=(k == KS - 1))

                    def ev(ps):
                        with nc.allow_low_precision(reason="bf16"):
                            nc.vector.tensor_tensor(
                                v[:, tk_i, 0:512], ps[:], bvb[:, 0:512], Alu.add)
                    group(thunks, sc, mm, ev)

                    def mm8(ps, k):
                        nc.tensor.matmul(ps[:, 0:8], xsl[:, k, :], Wv[:, k, 512:520],
                                         start=(k == 0), stop=(k == KS - 1))

                    def ev8(ps):
                        with nc.allow_low_precision(reason="bf16"):
                            nc.vector.tensor_tensor(
                                v[:, tk_i, 512:520], ps[:, 0:8], bvb[:, 512:520],
                                Alu.add)
                    group(thunks, sc, mm8, ev8)

                for m in range(MS):
                    k_mm(m)
                for ts in range(TSUB):
                    v_mm(ts)
                return thunks

            def q_thunks(ch, xt):
                qt = stream.tile([128, MS, TCH], bf16, tag="qt", bufs=4, name="qt")
                thunks = []
                sc = f"A{ch}"

                def q_mm(m):
                    def mm(ps, k):
                        nc.tensor.matmul(ps[:], Wq[:, k, m * 128:(m + 1) * 128],
                                         xt[:, k, :], start=(k == 0),
                                         stop=(k == KS - 1))

                    def ev(ps):
                        with nc.allow_low_precision(reason="bf16"):
                            nc.vector.tensor_scalar(
                                out=qt[:, m, :], in0=ps[:],
                                scalar1=bq[:, m:m + 1], scalar2=None, op0=Alu.add)
                    group(thunks, sc, mm, ev)

                for m in range(MS):
                    q_mm(m)
                return qt, thunks

            fill_q = []
            proj_q = []
            allow_proj = [False]
            popped = [0]
            queued = [0]

            def drain(n):
                while n > 0 and fill_q:
                    fill_q.pop(0)()
                    popped[0] += 1
                    n -= 1
                while n > 0 and allow_proj[0] and proj_q:
                    proj_q.pop(0)()
                    n -= 1

            def drain_until(mark):
                while popped[0] < mark and fill_q:
                    fill_q.pop(0)()
                    popped[0] += 1

            def queue(thunks):
                fill_q.extend(thunks)
                queued[0] += len(thunks)

            def emit_pair(ch, qt, yt, p, per_drain, mm_bcast=False):
                nblk = (ch + 1) * TSUB
                hA, hB = 2 * p, 2 * p + 1
                sc = f"attn{ch}"
                psy = {h: pY.tile([128, 512], f32, tag="pY", name=f"psy{h}")
                       for h in (hA, hB)}
                for g in range(nblk // 2):
                    i0, i1 = 2 * g, 2 * g + 1
                    pss = {}
                    # S matmuls: explicit row tile_position for pair concurrency
                    with nc.named_scope(sc):
                        for half, i in ((0, i0), (1, i1)):
                            dk = i - ch * TSUB
                            vs = 128 * dk if dk > 0 else 0
                            for h in (hA, hB):
                                pb = (h % 2) * 64
                                hm = h // 2
                                if half == 0:
                                    pss[h] = pS.tile([128, 1024], f32, tag="pS",
                                                     name=f"pss{h}")
                                nc.tensor.matmul(
                                    pss[h][:, half * TCH + vs: (half + 1) * TCH],
                                    kT[pb:pb + 64, hm, i * 128:(i + 1) * 128],
                                    qt[pb:pb + 64, hm, vs:TCH],
                                    start=True, stop=True,
                                    tile_position=(pb, 0))
                    drain(per_drain)
                    es = {}
                    with nc.named_scope(sc):
                        for h in (hA, hB):
                            es[h] = esp.tile([128, 2 * TCH], bf16, tag="es",
                                             name=f"es{h}")
                            dk1 = i1 - ch * TSUB
                            if dk1 <= 0:
                                with nc.allow_low_precision(reason="bf16"):
                                    nc.scalar.activation(es[h][:, :], pss[h][:, :],
                                                         Act.Exp, scale=float(SCALE))
                            else:
                                for half, i in ((0, i0), (1, i1)):
                                    dk = i - ch * TSUB
                                    vs = 128 * dk if dk > 0 else 0
                                    sl = slice(half * TCH + vs, (half + 1) * TCH)
                                    with nc.allow_low_precision(reason="bf16"):
                                        nc.scalar.activation(es[h][:, sl], pss[h][:, sl],
                                                             Act.Exp, scale=float(SCALE))
                            for half, i in ((0, i0), (1, i1)):
                                dk = i - ch * TSUB
                                if dk >= 0:
                                    vs = 128 * dk
                                    sl = slice(half * TCH + vs, half * TCH + vs + 128)
                                    with nc.allow_low_precision(reason="bf16"):
                                        nc.vector.tensor_tensor(
                                            es[h][:, sl], es[h][:, sl], tri[:], Alu.mult)
                        for half, i in ((0, i0), (1, i1)):
                            dk = i - ch * TSUB
                            vs = 128 * dk if dk > 0 else 0
                            for h in (hA, hB):
                                nc.tensor.matmul(
                                    psy[h][0:65, vs:TCH],
                                    v[:, i, h * 65:(h + 1) * 65],
                                    es[h][:, half * TCH + vs: (half + 1) * TCH],
                                    start=(i == 0), stop=(i == nblk - 1))
                    drain(per_drain)
                # normalization: reciprocal row -> DRAM bounce -> broadcast DMA
                # (async on the DMA engines; a PE matmul broadcast here would
                # stall the in-order PE queue on the DVE reciprocal)
                with nc.named_scope(f"norm{ch}"):
                    for h in (hA, hB):
                        pb = (h % 2) * 64
                        hm = h // 2
                        lrow = small.tile([1, TCH], f32, tag="lrow", name=f"lrow{h}")
                        nc.vector.tensor_copy(lrow[0:1, :], psy[h][64:65, :])
                        rrow = small.tile([1, TCH], f32, tag="rrow")
                        nc.vector.reciprocal_approx_fast(rrow[0:1, :], lrow[0:1, :])
                        with nc.allow_low_precision(reason="bf16"):
                            # plain evac first so the psum bank frees fast
                            nc.vector.tensor_copy(yt[pb:pb + 64, hm, :], psy[h][0:64, :])
                        if mm_bcast:
                            # low-latency path for the final pair only: K=1 fp32
                            # matmul broadcast (no DRAM roundtrip on the tail;
                            # nothing left in the PE queue to stall)
                            psb = pA.tile([128, 512], f32, tag="pA", name=f"psb{h}")
                            nc.tensor.matmul(psb[0:64, :], ones1[0:1, 0:64],
                                             rrow[0:1, :], start=True, stop=True)
                            with nc.allow_low_precision(reason="bf16"):
                                nc.vector.tensor_tensor(
                                    yt[pb:pb + 64, hm, :], yt[pb:pb + 64, hm, :],
                                    psb[0:64, :], Alu.mult)
                        else:
                            drow = dram.tile([1, TCH], f32, tag="drow", name=f"drow{h}")
                            nc.sync.dma_start(drow[:], rrow[:])
                            rbc = small.tile([128, TCH], f32, tag="rbc", name=f"rbc{h}")
                            nc.sync.dma_start(rbc[pb:pb + 64, :],
                                              drow[0:1, :].to_broadcast([64, TCH]))
                            with nc.allow_low_precision(reason="bf16"):
                                nc.vector.tensor_tensor(
                                    yt[pb:pb + 64, hm, :], yt[pb:pb + 64, hm, :],
                                    rbc[pb:pb + 64, :], Alu.mult)

            def proj_thunks(ch, yt):
                tq0 = ch * TCH
                thunks = []
                sc = f"proj{ch}"
                for mt in range(TSUB):
                    for n in range(C // 512):
                        st = {}

                        def mk(k, mt=mt, n=n, st=st):
                            def t():
                                with nc.named_scope(sc):
                                    if k == 3:
                                        st["ps"] = pA.tile([128, 512], f32, tag="pA",
                                                           name="psp")
                                    nc.tensor.matmul(
                                        st["ps"][:],
                                        yt[:, k, mt * 128:(mt + 1) * 128],
                                        Wp[:, k, n * 512:(n + 1) * 512],
                                        start=(k == 3), stop=(k == MS - 2))
                                    if k == MS - 2:
                                        ot = small.tile([128, 512], f32, tag="ot")
                                        nc.vector.tensor_copy(ot[:], st["ps"][:])
                                        nc.sync.dma_start(
                                            out_d.ap()[tq0 + mt * 128:
                                                       tq0 + (mt + 1) * 128,
                                                       n * 512:(n + 1) * 512],
                                            ot[:])
                            return t
                        thunks.extend(mk(k) for k in (3, 0, 1, 2))
                return thunks

            # ---- main schedule ----
            # chunk-0 inputs first so the PE can start ASAP: interleave the
            # xt(0) and Wk k-slices (the first k-matmul needs slice 0 of both)
            xt0 = stream.tile([128, KS, TCH], bf16, tag="xt", bufs=4, name="xt")
            x0src = xT_d.ap()[:, 0:TCH].rearrange("(ko p) t -> p ko t", p=128)
            wksrc = Wk_d.ap().rearrange("(ko p) m -> p ko m", p=128)
            for kk in range(KS):
                nc.sync.dma_start(xt0[:, kk:kk + 1, :], x0src[:, kk:kk + 1, :])
                nc.sync.dma_start(Wk[:, kk:kk + 1, :], wksrc[:, kk:kk + 1, :])
            nc.sync.dma_start(Wv[:], Wv_d.ap().rearrange("(ko p) m -> p ko m", p=128))
            nc.sync.dma_start(Wq[:], Wq_d.ap().rearrange("(ko p) m -> p ko m", p=128))
            nc.sync.dma_start(Wp[:], Wp_d.ap().rearrange("(m p) e -> p m e", p=128))

            # bvb broadcast setup (PE warms up while the weight DMAs stream)
            ps = pA.tile([128, 512], f32, tag="pA")
            nc.tensor.matmul(ps[:, :], ones1[0:1, :], bv_row[0:1, 0:512],
                             start=True, stop=True)
            nc.scalar.copy(bvb[:, 0:512], ps[:, :])
            ps = pA.tile([128, 512], f32, tag="pA")
            nc.tensor.matmul(ps[:, 0:8], ones1[0:1, :], bv_row[0:1, 512:520],
                             start=True, stop=True)
            nc.scalar.copy(bvb[:, 512:520], ps[:, 0:8])

            # chunk 0 phase A emitted directly
            for t in kv_thunks(0, xt0):
                t()
            qt0, q0 = q_thunks(0, xt0)
            for t in q0:
                t()

            # queue phase A of all later chunks up front: kv then q per chunk
            qt_tab = {0: qt0}
            q_marker = {0: 0}
            for c in range(1, NCH):
                xtc = make_xt(c)
                queue(kv_thunks(c, xtc))
                qtc, qc = q_thunks(c, xtc)
                queue(qc)
                qt_tab[c] = qtc
                q_marker[c] = queued[0]

            def n_drains(c):
                return 2 * 4 * ((c + 1) * TSUB // 2)

            # Pacing: land A(ch+1) ~60% into window ch (early chunks) so the
            # exp of later chunks can start sooner; spread A(3) over all of
            # window 2; drain the deferred proj fillers only during chunk 3,
            # where exp dominates and the PE would otherwise go sparse (HAM
            # re-throttles the PE clock on sparse activity).
            yt_tab = {}
            for ch in range(NCH):
                drain_until(q_marker[ch])
                yt = stream.tile([128, MS, TCH], bf16, tag="yt", bufs=4, name="yt")
                yt_tab[ch] = yt
                if ch == NCH - 1:
                    allow_proj[0] = True
                    todo = len(fill_q) + len(proj_q)
                    pts = n_drains(ch)
                else:
                    nxt = q_marker.get(ch + 1, queued[0])
                    todo = nxt - popped[0]
                    frac = 0.6 if ch < 2 else 1.0
                    pts = max(1, int(n_drains(ch) * frac))
                per_drain = max(1, -(-todo // max(pts, 1)))
                for p in (3, 0, 1, 2):
                    emit_pair(ch, qt_tab[ch], yt, p, per_drain,
                              mm_bcast=(ch == NCH - 1 and p == 2))
                if ch < NCH - 1:
                    proj_q.extend(proj_thunks(ch, yt))
            drain(len(fill_q) + len(proj_q))
            for t in proj_thunks(NCH - 1, yt_tab[NCH - 1]):
                t()

    nc.compile()
    return nc


def _get_nc():
    if "nc" not in _CACHE:
        _CACHE["nc"] = _build_nc()
    return _CACHE["nc"]


def kernel(x, W_qkv, b_qkv, W_proj, b_proj):
    global LAST_RESULTS
    from concourse.bass_utils import run_bass_kernel_spmd

    x = np.asarray(x, dtype=np.float32)
    W_qkv = np.asarray(W_qkv, dtype=np.float32)
    b_qkv = np.asarray(b_qkv, dtype=np.float32)
    W_proj = np.asarray(W_proj, dtype=np.float32)
    b_proj = np.asarray(b_proj, dtype=np.float32)

    nc = _get_nc()

    tri = np.tril(np.ones((128, 128), dtype=np.float32)).T.copy()  # tri[p,f]=1 iff p<=f

    in_maps = []
    for j in range(N_CORES):
        bi, g = j // 2, j % 2
        c0 = g * HCOLS
        Wv_h = W_qkv[:, 2 * C + c0:2 * C + c0 + HCOLS]
        bv_h = b_qkv[2 * C + c0:2 * C + c0 + HCOLS]
        Wv_aug = np.zeros((C, VAUG), dtype=np.float32)
        bv_aug = np.zeros((1, VAUG), dtype=np.float32)
        for h in range(HPC):
            Wv_aug[:, h * 65:h * 65 + 64] = Wv_h[:, h * 64:(h + 1) * 64]
            bv_aug[0, h * 65:h * 65 + 64] = bv_h[h * 64:(h + 1) * 64]
            bv_aug[0, h * 65 + 64] = 1.0
        bf16 = np.float16
        in_maps.append({
            "xT": x[bi].T.astype(bf16),
            "Wq": W_qkv[:, c0:c0 + HCOLS].astype(bf16),
            "Wk": W_qkv[:, C + c0:C + c0 + HCOLS].astype(bf16),
            "Wv": Wv_aug.astype(bf16),
            "Wp": W_proj[c0:c0 + HCOLS, :].astype(bf16),
            "bq": np.ascontiguousarray(b_qkv[c0:c0 + HCOLS]),
            "bk": np.ascontiguousarray(b_qkv[C + c0:C + c0 + HCOLS]),
            "bv": bv_aug,
            "TRI": tri.astype(bf16),
        })

    res = run_bass_kernel_spmd(nc, in_maps, list(range(N_CORES)))
    LAST_RESULTS = res

    out = np.empty((B, T, C), dtype=np.float32)
    for bi in range(B):
        out[bi] = res.results[2 * bi]["out"] + res.results[2 * bi + 1]["out"] + b_proj
    return out
